# revision 9
# baseline (speedup 1.0000x reference)
"""Trainium2 Bass kernel for padded-LSTM + CELU + projection (nn_Model_11888469476019).

Model (per reference):
  xp = pad(x, (2,3) on time, value=-0.5)            # [B, T=517, 32]
  gates z = xp @ W_ih.T + h @ W_hh.T + (b_ih+b_hh)  # LSTM, PyTorch gate order i,f,g,o
  c' = sigmoid(f)*c + sigmoid(i)*tanh(g)
  h' = sigmoid(o)*tanh(c')
  out[t] = celu(h') + xp[t] @ proj_w.T + proj_b,  kept for t in [2, 514)

Sharding: pure data-parallel, batch 4096 -> 512 per core across 8 cores.

Device design (per core, batch 512 = 4 chunks of 128):
  - x is converted to bf16 on host; on device each 4-timestep block
    [512b, 128(t,f)] is xbar-DMA-transposed to feature-major [128, 512].
  - Per step, a persistent "R" tile [112, 512] bf16 holds the merged matmul
    stationary operand: rows 0-31 x_t (feature major), row 32 ones (bias row),
    rows 33-63 zero, rows 64-111 w2 = 2*h (feature major).
  - Gate matmuls: out G[128b, 192] per chunk = R_chunk.T @ WG, K=112.
    WG rows: [W_ih.T; b_ih+b_hh; 0; 0.5*W_hh.T], g-gate cols pre-scaled by 2.
  - All-tanh formulation (exp+tanh share one ACT table set):
      T = tanh(0.5 * z)  (one ACT op; for g-gate: z pre-scaled 2x -> tanh(z_g))
      U  = (t_i + 1) * t_g            # = 2*sigmoid(i)*tanh(g)
      M4 = (t_f + 1) * C2             # C2 = 2c state; = 4*sigmoid(f)*c
      C2' = 0.5*M4 + U                # = 2c'
      TC = tanh(0.5 * C2')            # = tanh(c')
      w2 = (t_o + 1) * TC             # = 2h'  (0.5 folded into W_hh)
  - w2 (batch-major) is PE-transposed back to feature-major into R for the
    next step's matmul.
  - Output path (batched over 4 steps): E = exp(0.5*w2), r = max(0.5*w2, 0),
    celu = min(E-1, r), out = celu + proj (proj from its own matmul, psum).
"""
import os
import numpy as np
import ml_dtypes

B_TOT, S_LEN, INP, HID = 4096, 512, 32, 48
NCORES = 8
B_CORE = B_TOT // NCORES  # 512
PAD_L = 2
T_STEPS = S_LEN + PAD_L   # 514 steps; trailing pads never affect the output
NG = 4 * HID              # 192
PAD_VAL = -0.5
NPBF16 = ml_dtypes.bfloat16

_BUILT = {}


def _build_nc():
    """Build (and cache) the Bass program for one core."""
    if "nc" in _BUILT:
        return _BUILT["nc"]

    from contextlib import ExitStack

    import concourse.bacc as bacc
    import concourse.bass as bass
    import concourse.mybir as mybir
    import concourse.tile as tile

    F32 = mybir.dt.float32
    BF16 = mybir.dt.bfloat16
    AF = mybir.ActivationFunctionType
    ALU = mybir.AluOpType

    nc = bacc.Bacc("TRN2", target_bir_lowering=False, debug=False,
                   enable_asserts=False)

    xt = nc.dram_tensor("xt", [B_CORE, S_LEN * INP], BF16, kind="ExternalInput")
    wg = nc.dram_tensor("wg", [112, NG + HID], BF16, kind="ExternalInput")
    ident_d = nc.dram_tensor("ident", [128, 128], BF16, kind="ExternalInput")
    out_d = nc.dram_tensor("out", [B_CORE, S_LEN, HID], F32, kind="ExternalOutput")

    with tile.TileContext(nc) as tc, ExitStack() as ctx:
        consts = ctx.enter_context(tc.tile_pool(name="consts", bufs=1))
        xch = ctx.enter_context(tc.tile_pool(name="xch", bufs=4))
        sp = ctx.enter_context(tc.tile_pool(name="sp", bufs=2))
        op = ctx.enter_context(tc.tile_pool(name="op", bufs=2))
        gp = ctx.enter_context(tc.tile_pool(name="gp", bufs=2, space="PSUM"))
        wtp = ctx.enter_context(tc.tile_pool(name="wtp", bufs=2, space="PSUM"))
        pp = ctx.enter_context(tc.tile_pool(name="pp", bufs=2, space="PSUM"))

        WG = consts.tile([112, NG + HID], BF16)
        nc.sync.dma_start(WG[:], wg[:])
        ident = consts.tile([128, 128], BF16)
        nc.sync.dma_start(ident[:], ident_d[:])

        # Persistent state tiles (double-buffered by step parity).
        R = [consts.tile([112, B_CORE], BF16, name=f"R{i}") for i in range(2)]
        C2 = [consts.tile([128, 4, HID], BF16, name=f"C2{i}") for i in range(2)]
        # w2 ring: 8 step-slots so a 4-step output group reads a contiguous,
        # non-wrapping [*, 4, 4, 48] block (slots (t-2)%8).
        W2R = consts.tile([128, 8, 4, HID], BF16, name="W2R")

        for i in range(2):
            nc.gpsimd.memset(R[i][32:64, :], 0.0)
            nc.gpsimd.memset(R[i][32:33, :], 1.0)
        nc.gpsimd.memset(R[0][64:112, :], 0.0)   # h0 = 0
        nc.vector.memset(C2[0][:], 0.0)          # c0 = 0

        chunk = None
        P = None
        for t in range(T_STEPS):
            Rc, Rn = R[t % 2], R[(t + 1) % 2]
            C2c, C2n = C2[t % 2], C2[(t + 1) % 2]
            s8 = (t - PAD_L) % 8           # w2 ring slot
            s2 = (t - PAD_L) % 2           # slot within output group

            # --- x supply ---
            if t < PAD_L:
                nc.gpsimd.memset(Rc[0:32, :], PAD_VAL)
            else:
                u = t - PAD_L              # x timestep index 0..511
                if u % 4 == 0:
                    chunk = xch.tile([128, B_CORE], BF16, tag="chunk")
                    nc.sync.dma_start_transpose(
                        chunk[:], xt[:, u * INP:(u + 4) * INP])
                cs = (u % 4) * INP
                nc.sync.dma_start(Rc[0:32, :], chunk[cs:cs + 32, :])

            # --- gate (+proj) matmuls ---
            G = gp.tile([128, 4, 256], F32, tag="G")
            if t >= PAD_L and s2 == 0:
                P = pp.tile([128, 2, 4, HID], F32, tag="P")
            for c in range(4):
                lhsT = Rc[:, c * 128:(c + 1) * 128]
                nc.tensor.matmul(G[:, c, 0:NG], lhsT=lhsT, rhs=WG[:, 0:NG],
                                 start=True, stop=True)
                if t >= PAD_L:
                    nc.tensor.matmul(P[:, s2, c, :], lhsT=lhsT,
                                     rhs=WG[:, NG:NG + HID],
                                     start=True, stop=True)

            # --- gate activations: T = tanh(0.5 * z) ---
            S = sp.tile([128, 4, NG], BF16, tag="S")
            nc.scalar.activation(S[:], G[:, :, 0:NG], AF.Tanh, scale=0.5)
            t_i = S[:, :, 0:48]
            t_f = S[:, :, 48:96]
            t_g = S[:, :, 96:144]
            t_o = S[:, :, 144:192]

            # --- cell update ---
            M4 = sp.tile([128, 4, HID], BF16, tag="M4")
            nc.vector.scalar_tensor_tensor(M4[:], t_f, 1.0, C2c[:],
                                           op0=ALU.add, op1=ALU.mult)
            U = sp.tile([128, 4, HID], BF16, tag="U")
            nc.vector.scalar_tensor_tensor(U[:], t_i, 1.0, t_g,
                                           op0=ALU.add, op1=ALU.mult)
            nc.vector.scalar_tensor_tensor(C2n[:], M4[:], 0.5, U[:],
                                           op0=ALU.mult, op1=ALU.add)
            TC = sp.tile([128, 4, HID], BF16, tag="TC")
            nc.scalar.activation(TC[:], C2n[:], AF.Tanh, scale=0.5)
            w2 = W2R[:, s8, :, :]
            nc.vector.scalar_tensor_tensor(w2, t_o, 1.0, TC[:],
                                           op0=ALU.add, op1=ALU.mult)

            # --- transpose w2 back to feature-major for next step ---
            wT = wtp.tile([48, B_CORE], BF16, tag="wT")
            for c in range(4):
                nc.tensor.transpose(wT[:, c * 128:(c + 1) * 128],
                                    W2R[:, s8, c, :], ident[:])
            nc.vector.tensor_copy(Rn[64:112, :], wT[:])

            # --- output path, batched per 2 steps ---
            if t >= PAD_L and s2 == 1:
                g0 = s8 - 1            # first slot of this group (even)
                wv = W2R[:, g0:g0 + 2, :, :]
                E = op.tile([128, 2, 4, HID], BF16, tag="E")
                nc.scalar.activation(E[:], wv, AF.Exp, scale=0.5)
                r = op.tile([128, 2, 4, HID], BF16, tag="r")
                nc.gpsimd.tensor_scalar(r[:], wv, 0.5, 0.0,
                                        op0=ALU.mult, op1=ALU.max)
                m = op.tile([128, 2, 4, HID], BF16, tag="m")
                nc.vector.scalar_tensor_tensor(m[:], E[:], 1.0, r[:],
                                               op0=ALU.subtract, op1=ALU.min)
                OT = op.tile([128, 2, 4, HID], F32, tag="OT")
                nc.vector.scalar_tensor_tensor(OT[:], m[:], 0.0, P[:],
                                               op0=ALU.add, op1=ALU.add)
                so = t - PAD_L - 1     # first output s-index of group
                for c in range(4):
                    nc.sync.dma_start(
                        out_d[c * 128:(c + 1) * 128, so:so + 2, :],
                        OT[:, :, c, :])

    nc.compile()
    _BUILT["nc"] = nc
    return nc


def _prep_weights(W_ih, W_hh, b_ih, b_hh, proj_w, proj_b):
    scale = np.ones((NG,), np.float32)
    scale[96:144] = 2.0  # g-gate pre-scale (tanh(0.5*2z) = tanh(z))
    Wg = np.zeros((112, NG + HID), np.float32)
    Wg[0:32, 0:NG] = W_ih.T * scale
    Wg[32, 0:NG] = (b_ih + b_hh) * scale
    Wg[64:112, 0:NG] = 0.5 * W_hh.T * scale   # w2 = 2h fold
    Wg[0:32, NG:] = proj_w.T
    Wg[32, NG:] = proj_b
    return Wg.astype(NPBF16)


def kernel(x, W_ih, W_hh, b_ih, b_hh, proj_w, proj_b):
    x = np.asarray(x, np.float32)
    Wg = _prep_weights(np.asarray(W_ih, np.float32), np.asarray(W_hh, np.float32),
                       np.asarray(b_ih, np.float32), np.asarray(b_hh, np.float32),
                       np.asarray(proj_w, np.float32), np.asarray(proj_b, np.float32))
    ident = np.eye(128, dtype=NPBF16)
    xbf = x.astype(NPBF16).reshape(B_TOT, S_LEN * INP)

    nc = _build_nc()
    from concourse import bass_utils

    in_maps = []
    for i in range(NCORES):
        in_maps.append({
            "xt": xbf[i * B_CORE:(i + 1) * B_CORE],
            "wg": Wg,
            "ident": ident,
        })
    res = bass_utils.run_bass_kernel_spmd(nc, in_maps, core_ids=list(range(NCORES)))
    out = np.concatenate([r["out"] for r in res.results], axis=0)
    return out


# revision 14
# speedup vs baseline: 1.1941x; 1.1941x over previous
"""Trainium2 Bass kernel for padded-LSTM + CELU + projection (nn_Model_11888469476019).

Model (per reference):
  xp = pad(x, (2,3) on time, value=-0.5)            # [B, T=517, 32]
  gates z = xp @ W_ih.T + h @ W_hh.T + (b_ih+b_hh)  # LSTM, PyTorch gate order i,f,g,o
  c' = sigmoid(f)*c + sigmoid(i)*tanh(g)
  h' = sigmoid(o)*tanh(c')
  out[t] = celu(h') + xp[t] @ proj_w.T + proj_b,  kept for t in [2, 514)

Sharding: pure data-parallel, batch 4096 -> 512 per core across 8 cores.

Device design (per core, batch 512 = 4 chunks of 128):
  - x is converted to bf16 on host; on device each 4-timestep block
    [512b, 128(t,f)] is xbar-DMA-transposed to feature-major [128, 512].
  - Per step, a persistent "R" tile [112, 512] bf16 holds the merged matmul
    stationary operand: rows 0-31 x_t (feature major), row 32 ones (bias row),
    rows 33-63 zero, rows 64-111 w2 = 2*h (feature major).
  - Gate matmuls: out G[128b, 192] per chunk = R_chunk.T @ WG, K=112.
    WG rows: [W_ih.T; b_ih+b_hh; 0; 0.5*W_hh.T], g-gate cols pre-scaled by 2.
  - All-tanh formulation (exp+tanh share one ACT table set):
      T = tanh(0.5 * z)  (one ACT op; for g-gate: z pre-scaled 2x -> tanh(z_g))
      U  = (t_i + 1) * t_g            # = 2*sigmoid(i)*tanh(g)
      M4 = (t_f + 1) * C2             # C2 = 2c state; = 4*sigmoid(f)*c
      C2' = 0.5*M4 + U                # = 2c'
      TC = tanh(0.5 * C2')            # = tanh(c')
      w2 = (t_o + 1) * TC             # = 2h'  (0.5 folded into W_hh)
  - w2 (batch-major) is PE-transposed back to feature-major into R for the
    next step's matmul.
  - Output path (batched over 4 steps): E = exp(0.5*w2), r = max(0.5*w2, 0),
    celu = min(E-1, r), out = celu + proj (proj from its own matmul, psum).
"""
import os
import numpy as np
import ml_dtypes

B_TOT, S_LEN, INP, HID = 4096, 512, 32, 48
NCORES = 8
B_CORE = B_TOT // NCORES  # 512
PAD_L = 2
T_STEPS = S_LEN + PAD_L   # 514 steps; trailing pads never affect the output
NG = 4 * HID              # 192
PAD_VAL = -0.5
NPBF16 = ml_dtypes.bfloat16

_BUILT = {}


def _build_nc():
    """Build (and cache) the Bass program for one core."""
    if "nc" in _BUILT:
        return _BUILT["nc"]

    from contextlib import ExitStack

    import concourse.bacc as bacc
    import concourse.bass as bass
    import concourse.mybir as mybir
    import concourse.tile as tile

    F32 = mybir.dt.float32
    BF16 = mybir.dt.bfloat16
    AF = mybir.ActivationFunctionType
    ALU = mybir.AluOpType

    nc = bacc.Bacc("TRN2", target_bir_lowering=False, debug=False,
                   enable_asserts=False)

    xt = nc.dram_tensor("xt", [B_CORE, S_LEN * INP], BF16, kind="ExternalInput")
    wg = nc.dram_tensor("wg", [112, NG + HID], BF16, kind="ExternalInput")
    ident_d = nc.dram_tensor("ident", [128, 128], BF16, kind="ExternalInput")
    out_d = nc.dram_tensor("out", [B_CORE, S_LEN, HID], F32, kind="ExternalOutput")

    with tile.TileContext(nc) as tc, ExitStack() as ctx:
        consts = ctx.enter_context(tc.tile_pool(name="consts", bufs=1))
        xch = ctx.enter_context(tc.tile_pool(name="xch", bufs=4))
        sp = ctx.enter_context(tc.tile_pool(name="sp", bufs=2))
        op = ctx.enter_context(tc.tile_pool(name="op", bufs=2))
        gp = ctx.enter_context(tc.tile_pool(name="gp", bufs=2, space="PSUM"))
        wtp = ctx.enter_context(tc.tile_pool(name="wtp", bufs=1, space="PSUM"))
        pp = ctx.enter_context(tc.tile_pool(name="pp", bufs=2, space="PSUM"))

        WG = consts.tile([112, NG + HID], BF16)
        nc.sync.dma_start(WG[:], wg[:])
        ident = consts.tile([128, 128], BF16)
        nc.sync.dma_start(ident[:], ident_d[:])

        # Persistent per-half state tiles (halves = batch 0-255 / 256-511,
        # chunks 0-1 / 2-3). Two independent recurrences whose dependency
        # cycles interleave on the engines.
        R = [[consts.tile([112, 256], BF16, name=f"R{h}{i}") for i in range(2)]
             for h in range(2)]
        C2 = [[consts.tile([128, 2, HID], BF16, name=f"C2{h}{i}") for i in range(2)]
              for h in range(2)]
        W2R = [consts.tile([128, 8, 2, HID], BF16, name=f"W2R{h}") for h in range(2)]

        for h in range(2):
            for i in range(2):
                nc.gpsimd.memset(R[h][i][32:64, :], 0.0)
                nc.gpsimd.memset(R[h][i][32:33, :], 1.0)
            nc.gpsimd.memset(R[h][0][64:112, :], 0.0)   # h0 = 0
            nc.vector.memset(C2[h][0][:], 0.0)          # c0 = 0

        chunk = None
        P = None
        for t in range(T_STEPS):
            Rc = [R[h][t % 2] for h in range(2)]
            Rn = [R[h][(t + 1) % 2] for h in range(2)]
            C2c = [C2[h][t % 2] for h in range(2)]
            C2n = [C2[h][(t + 1) % 2] for h in range(2)]
            s8 = (t - PAD_L) % 8           # w2 ring slot
            s2 = (t - PAD_L) % 2           # slot within output group

            # --- x supply ---
            if t < PAD_L:
                for h in range(2):
                    nc.gpsimd.memset(Rc[h][0:32, :], PAD_VAL)
            else:
                u = t - PAD_L              # x timestep index 0..511
                if u % 4 == 0:
                    chunk = xch.tile([128, B_CORE], BF16, tag="chunk")
                    nc.sync.dma_start_transpose(
                        chunk[:], xt[:, u * INP:(u + 4) * INP])
                cs = (u % 4) * INP
                for h in range(2):
                    nc.sync.dma_start(Rc[h][0:32, :],
                                      chunk[cs:cs + 32, h * 256:(h + 1) * 256])

            # --- gate (+proj) matmuls ---
            G = [gp.tile([128, 2, 256], F32, tag=f"G{h}", name=f"Gt{h}")
                 for h in range(2)]
            if t >= PAD_L and s2 == 0:
                Pprev = P
                P = pp.tile([128, 2, 4, HID], F32, tag="P", name="Pt")
            for h in range(2):
                for cc in range(2):
                    lhsT = Rc[h][:, cc * 128:(cc + 1) * 128]
                    nc.tensor.matmul(G[h][:, cc, 0:NG], lhsT=lhsT,
                                     rhs=WG[:, 0:NG], start=True, stop=True)
                    if t >= PAD_L:
                        nc.tensor.matmul(P[:, s2, h * 2 + cc, :], lhsT=lhsT,
                                         rhs=WG[:, NG:NG + HID],
                                         start=True, stop=True)

            # --- gate activations + cell update, per half ---
            S = [sp.tile([128, 2, NG], BF16, tag=f"S{h}", name=f"St{h}")
                 for h in range(2)]
            M4 = [sp.tile([128, 2, HID], BF16, tag=f"M4{h}", name=f"M4t{h}")
                  for h in range(2)]
            U = [sp.tile([128, 2, HID], BF16, tag=f"U{h}", name=f"Ut{h}")
                 for h in range(2)]
            TC = [sp.tile([128, 2, HID], BF16, tag=f"TC{h}", name=f"TCt{h}")
                  for h in range(2)]
            for h in range(2):
                nc.scalar.activation(S[h][:], G[h][:, :, 0:NG], AF.Tanh, scale=0.5)
                t_i = S[h][:, :, 0:48]
                t_f = S[h][:, :, 48:96]
                t_g = S[h][:, :, 96:144]
                t_o = S[h][:, :, 144:192]
                nc.vector.scalar_tensor_tensor(M4[h][:], t_f, 1.0, C2c[h][:],
                                               op0=ALU.add, op1=ALU.mult)
                nc.vector.scalar_tensor_tensor(U[h][:], t_i, 1.0, t_g,
                                               op0=ALU.add, op1=ALU.mult)
                nc.vector.scalar_tensor_tensor(C2n[h][:], M4[h][:], 0.5, U[h][:],
                                               op0=ALU.mult, op1=ALU.add)
                nc.scalar.activation(TC[h][:], C2n[h][:], AF.Tanh, scale=0.5)
                w2 = W2R[h][:, s8, :, :]
                nc.vector.scalar_tensor_tensor(w2, t_o, 1.0, TC[h][:],
                                               op0=ALU.add, op1=ALU.mult)

                # transpose w2 back to feature-major for next step
                wT = wtp.tile([48, 256], BF16, tag=f"wT{h}", name=f"wTt{h}")
                for cc in range(2):
                    nc.tensor.transpose(wT[:, cc * 128:(cc + 1) * 128],
                                        W2R[h][:, s8, cc, :], ident[:])
                nc.vector.tensor_copy(Rn[h][64:112, :], wT[:])

            # --- output path: E/r/m batched per 4 steps, +proj per P tile ---
            if t >= PAD_L and (t - PAD_L) % 4 == 3:
                g0 = s8 - 3            # first slot of this 4-step group
                so = t - PAD_L - 3     # first output s-index of group
                for h in range(2):
                    wv = W2R[h][:, g0:g0 + 4, :, :]
                    E = op.tile([128, 4, 2, HID], BF16, tag=f"E{h}", name=f"Et{h}")
                    nc.scalar.activation(E[:], wv, AF.Exp, scale=0.5)
                    r = op.tile([128, 4, 2, HID], BF16, tag=f"r{h}", name=f"rt{h}")
                    nc.gpsimd.tensor_scalar(r[:], wv, 0.5, 0.0,
                                            op0=ALU.mult, op1=ALU.max)
                    m = op.tile([128, 4, 2, HID], BF16, tag=f"m{h}", name=f"mt{h}")
                    nc.vector.scalar_tensor_tensor(m[:], E[:], 1.0, r[:],
                                                   op0=ALU.subtract, op1=ALU.min)
                    OT = op.tile([128, 4, 2, HID], F32, tag=f"OT{h}", name=f"OTt{h}")
                    for gi, Pt in enumerate((Pprev, P)):
                        nc.vector.scalar_tensor_tensor(
                            OT[:, gi * 2:gi * 2 + 2, :, :],
                            m[:, gi * 2:gi * 2 + 2, :, :], 0.0,
                            Pt[:, :, h * 2:h * 2 + 2, :],
                            op0=ALU.add, op1=ALU.add)
                    for cc in range(2):
                        c = h * 2 + cc
                        nc.sync.dma_start(
                            out_d[c * 128:(c + 1) * 128, so:so + 4, :],
                            OT[:, :, cc, :])

    nc.compile()
    _BUILT["nc"] = nc
    return nc


def _prep_weights(W_ih, W_hh, b_ih, b_hh, proj_w, proj_b):
    scale = np.ones((NG,), np.float32)
    scale[96:144] = 2.0  # g-gate pre-scale (tanh(0.5*2z) = tanh(z))
    Wg = np.zeros((112, NG + HID), np.float32)
    Wg[0:32, 0:NG] = W_ih.T * scale
    Wg[32, 0:NG] = (b_ih + b_hh) * scale
    Wg[64:112, 0:NG] = 0.5 * W_hh.T * scale   # w2 = 2h fold
    Wg[0:32, NG:] = proj_w.T
    Wg[32, NG:] = proj_b
    return Wg.astype(NPBF16)


def kernel(x, W_ih, W_hh, b_ih, b_hh, proj_w, proj_b):
    x = np.asarray(x, np.float32)
    Wg = _prep_weights(np.asarray(W_ih, np.float32), np.asarray(W_hh, np.float32),
                       np.asarray(b_ih, np.float32), np.asarray(b_hh, np.float32),
                       np.asarray(proj_w, np.float32), np.asarray(proj_b, np.float32))
    ident = np.eye(128, dtype=NPBF16)
    xbf = x.astype(NPBF16).reshape(B_TOT, S_LEN * INP)

    nc = _build_nc()
    from concourse import bass_utils

    in_maps = []
    for i in range(NCORES):
        in_maps.append({
            "xt": xbf[i * B_CORE:(i + 1) * B_CORE],
            "wg": Wg,
            "ident": ident,
        })
    res = bass_utils.run_bass_kernel_spmd(nc, in_maps, core_ids=list(range(NCORES)))
    out = np.concatenate([r["out"] for r in res.results], axis=0)
    return out


# revision 26
# speedup vs baseline: 1.2185x; 1.0205x over previous
"""Trainium2 Bass kernel for padded-LSTM + CELU + projection (nn_Model_11888469476019).

Model (per reference):
  xp = pad(x, (2,3) on time, value=-0.5)            # [B, T=517, 32]
  gates z = xp @ W_ih.T + h @ W_hh.T + (b_ih+b_hh)  # LSTM, PyTorch gate order i,f,g,o
  c' = sigmoid(f)*c + sigmoid(i)*tanh(g)
  h' = sigmoid(o)*tanh(c')
  out[t] = celu(h') + xp[t] @ proj_w.T + proj_b,  kept for t in [2, 514)

Sharding: pure data-parallel, batch 4096 -> 512 per core across 8 cores.

Device design (per core, batch 512 = 4 chunks of 128):
  - x is converted to bf16 on host; on device each 4-timestep block
    [512b, 128(t,f)] is xbar-DMA-transposed to feature-major [128, 512].
  - Per step, a persistent "R" tile [112, 512] bf16 holds the merged matmul
    stationary operand: rows 0-31 x_t (feature major), row 32 ones (bias row),
    rows 33-63 zero, rows 64-111 w2 = 2*h (feature major).
  - Gate matmuls: out G[128b, 192] per chunk = R_chunk.T @ WG, K=112.
    WG rows: [W_ih.T; b_ih+b_hh; 0; 0.5*W_hh.T], g-gate cols pre-scaled by 2.
  - All-tanh formulation (exp+tanh share one ACT table set):
      T = tanh(0.5 * z)  (one ACT op; for g-gate: z pre-scaled 2x -> tanh(z_g))
      U  = (t_i + 1) * t_g            # = 2*sigmoid(i)*tanh(g)
      M4 = (t_f + 1) * C2             # C2 = 2c state; = 4*sigmoid(f)*c
      C2' = 0.5*M4 + U                # = 2c'
      TC = tanh(0.5 * C2')            # = tanh(c')
      w2 = (t_o + 1) * TC             # = 2h'  (0.5 folded into W_hh)
  - w2 (batch-major) is PE-transposed back to feature-major into R for the
    next step's matmul.
  - Output path (batched over 4 steps): E = exp(0.5*w2), r = max(0.5*w2, 0),
    celu = min(E-1, r), out = celu + proj (proj from its own matmul, psum).
"""
import os
import numpy as np
import ml_dtypes

B_TOT, S_LEN, INP, HID = 4096, 512, 32, 48
NCORES = 8
B_CORE = B_TOT // NCORES  # 512
PAD_L = 2
T_STEPS = S_LEN + PAD_L   # 514 steps; trailing pads never affect the output
NG = 4 * HID              # 192
PAD_VAL = -0.5
NPBF16 = ml_dtypes.bfloat16

_BUILT = {}


def _build_nc():
    """Build (and cache) the Bass program for one core."""
    if "nc" in _BUILT:
        return _BUILT["nc"]

    from contextlib import ExitStack

    import concourse.bacc as bacc
    import concourse.bass as bass
    import concourse.mybir as mybir
    import concourse.tile as tile

    F32 = mybir.dt.float32
    BF16 = mybir.dt.bfloat16
    AF = mybir.ActivationFunctionType
    ALU = mybir.AluOpType

    nc = bacc.Bacc("TRN2", target_bir_lowering=False, debug=False,
                   enable_asserts=False)

    xt = nc.dram_tensor("xt", [B_CORE, S_LEN * INP], BF16, kind="ExternalInput")
    wg = nc.dram_tensor("wg", [112, NG + HID], BF16, kind="ExternalInput")
    ident_d = nc.dram_tensor("ident", [128, 128], BF16, kind="ExternalInput")
    out_d = nc.dram_tensor("out", [B_CORE, S_LEN, HID], F32, kind="ExternalOutput")

    with tile.TileContext(nc) as tc, ExitStack() as ctx:
        consts = ctx.enter_context(tc.tile_pool(name="consts", bufs=1))
        xch = ctx.enter_context(tc.tile_pool(name="xch", bufs=4))
        sp = ctx.enter_context(tc.tile_pool(name="sp", bufs=2))
        op = ctx.enter_context(tc.tile_pool(name="op", bufs=2))
        gp = ctx.enter_context(tc.tile_pool(name="gp", bufs=1, space="PSUM"))
        wtp = ctx.enter_context(tc.tile_pool(name="wtp", bufs=1, space="PSUM"))
        pp = ctx.enter_context(tc.tile_pool(name="pp", bufs=2, space="PSUM"))

        WG = consts.tile([112, NG + HID], BF16)
        nc.sync.dma_start(WG[:], wg[:])
        ident = consts.tile([128, 128], BF16)
        nc.sync.dma_start(ident[:], ident_d[:])

        # Persistent per-half state tiles (halves = batch 0-255 / 256-511,
        # chunks 0-1 / 2-3). Two independent recurrences whose dependency
        # cycles interleave on the engines.
        R = [[consts.tile([112, 256], BF16, name=f"R{h}{i}") for i in range(2)]
             for h in range(2)]
        C2 = [[consts.tile([128, 2, HID], BF16, name=f"C2{h}{i}") for i in range(2)]
              for h in range(2)]
        W2R = [consts.tile([128, 8, 2, HID], BF16, name=f"W2R{h}") for h in range(2)]

        for h in range(2):
            for i in range(2):
                nc.gpsimd.memset(R[h][i][32:64, :], 0.0)
                nc.gpsimd.memset(R[h][i][32:33, :], 1.0)
            nc.gpsimd.memset(R[h][0][64:112, :], 0.0)   # h0 = 0
            nc.vector.memset(C2[h][0][:], 0.0)          # c0 = 0

        chunk = None
        P = None
        for t in range(T_STEPS):
            Rc = [R[h][t % 2] for h in range(2)]
            Rn = [R[h][(t + 1) % 2] for h in range(2)]
            C2c = [C2[h][t % 2] for h in range(2)]
            C2n = [C2[h][(t + 1) % 2] for h in range(2)]
            s8 = (t - PAD_L) % 8           # w2 ring slot
            s2 = (t - PAD_L) % 2
            s4 = (t - PAD_L) % 4           # slot within output group

            # --- x supply ---
            if t < PAD_L:
                for h in range(2):
                    nc.gpsimd.memset(Rc[h][0:32, :], PAD_VAL)
            else:
                u = t - PAD_L              # x timestep index 0..511
                if u % 4 == 0:
                    chunk = xch.tile([128, B_CORE], BF16, tag="chunk")
                    nc.sync.dma_start_transpose(
                        chunk[:], xt[:, u * INP:(u + 4) * INP])
                cs = (u % 4) * INP
                for h in range(2):
                    nc.sync.dma_start(Rc[h][0:32, :],
                                      chunk[cs:cs + 32, h * 256:(h + 1) * 256])

            # --- gate (+proj) matmuls ---
            G = [gp.tile([128, 2, 256], F32, tag=f"G{h}", name=f"Gt{h}")
                 for h in range(2)]
            if t >= PAD_L and s4 == 0:
                P = pp.tile([128, 4, 256], F32, tag="P", name="Pt")
            for h in range(2):
                for cc in range(2):
                    lhsT = Rc[h][:, cc * 128:(cc + 1) * 128]
                    nc.tensor.matmul(G[h][:, cc, 0:NG], lhsT=lhsT,
                                     rhs=WG[:, 0:NG], start=True, stop=True)
            if t >= PAD_L:
                for h in range(2):
                    for cc in range(2):
                        lhsT = Rc[h][:, cc * 128:(cc + 1) * 128]
                        c = h * 2 + cc
                        nc.tensor.matmul(P[:, s4, c * HID:(c + 1) * HID],
                                         lhsT=lhsT, rhs=WG[:, NG:NG + HID],
                                         start=True, stop=True)

            # --- gate activations + cell update, per half ---
            S = [sp.tile([128, 2, NG], BF16, tag=f"S{h}", name=f"St{h}")
                 for h in range(2)]
            M4 = [sp.tile([128, 2, HID], BF16, tag=f"M4{h}", name=f"M4t{h}")
                  for h in range(2)]
            U = [sp.tile([128, 2, HID], BF16, tag=f"U{h}", name=f"Ut{h}")
                 for h in range(2)]
            TC = [sp.tile([128, 2, HID], BF16, tag=f"TC{h}", name=f"TCt{h}")
                  for h in range(2)]
            for h in range(2):
                nc.scalar.activation(S[h][:], G[h][:, :, 0:NG], AF.Tanh, scale=0.5)
                t_i = S[h][:, :, 0:48]
                t_f = S[h][:, :, 48:96]
                t_g = S[h][:, :, 96:144]
                t_o = S[h][:, :, 144:192]
                nc.vector.scalar_tensor_tensor(M4[h][:], t_f, 1.0, C2c[h][:],
                                               op0=ALU.add, op1=ALU.mult)
                nc.vector.scalar_tensor_tensor(U[h][:], t_i, 1.0, t_g,
                                               op0=ALU.add, op1=ALU.mult)
                nc.vector.scalar_tensor_tensor(C2n[h][:], M4[h][:], 0.5, U[h][:],
                                               op0=ALU.mult, op1=ALU.add)
                nc.scalar.activation(TC[h][:], C2n[h][:], AF.Tanh, scale=0.5)
                w2 = W2R[h][:, s8, :, :]
                nc.vector.scalar_tensor_tensor(w2, t_o, 1.0, TC[h][:],
                                               op0=ALU.add, op1=ALU.mult)

                # transpose w2 back to feature-major for next step
                wT = wtp.tile([48, 256], BF16, tag=f"wT{h}", name=f"wTt{h}")
                for cc in range(2):
                    nc.tensor.transpose(wT[:, cc * 128:(cc + 1) * 128],
                                        W2R[h][:, s8, cc, :], ident[:])
                nc.vector.tensor_copy(Rn[h][64:112, :], wT[:])

            # --- output path: E/r/m batched per 4 steps, +proj per P tile ---
            if t >= PAD_L and (t - PAD_L) % 4 == 3:
                g0 = s8 - 3            # first slot of this 4-step group
                so = t - PAD_L - 3     # first output s-index of group
                for h in range(2):
                    wv = W2R[h][:, g0:g0 + 4, :, :]
                    E = op.tile([128, 4, 2, HID], BF16, tag=f"E{h}", name=f"Et{h}")
                    nc.scalar.activation(E[:], wv, AF.Exp, scale=0.5)
                    r = op.tile([128, 4, 2, HID], BF16, tag=f"r{h}", name=f"rt{h}")
                    nc.gpsimd.tensor_scalar(r[:], wv, 0.5, 0.0,
                                            op0=ALU.mult, op1=ALU.max)
                    m = op.tile([128, 4, 2, HID], BF16, tag=f"m{h}", name=f"mt{h}")
                    nc.vector.scalar_tensor_tensor(m[:], E[:], 1.0, r[:],
                                                   op0=ALU.subtract, op1=ALU.min)
                    OT = op.tile([128, 4, 2, HID], F32, tag=f"OT{h}", name=f"OTt{h}")
                    ps = P[:, :, h * 2 * HID:(h * 2 + 2) * HID].rearrange(
                        "p a (b c) -> p a b c", b=2)
                    nc.vector.scalar_tensor_tensor(OT[:], m[:], 0.0, ps,
                                                   op0=ALU.add, op1=ALU.add)
                    for cc in range(2):
                        c = h * 2 + cc
                        nc.sync.dma_start(
                            out_d[c * 128:(c + 1) * 128, so:so + 4, :],
                            OT[:, :, cc, :])

    nc.compile()
    _BUILT["nc"] = nc
    return nc


def _prep_weights(W_ih, W_hh, b_ih, b_hh, proj_w, proj_b):
    scale = np.ones((NG,), np.float32)
    scale[96:144] = 2.0  # g-gate pre-scale (tanh(0.5*2z) = tanh(z))
    Wg = np.zeros((112, NG + HID), np.float32)
    Wg[0:32, 0:NG] = W_ih.T * scale
    Wg[32, 0:NG] = (b_ih + b_hh) * scale
    Wg[64:112, 0:NG] = 0.5 * W_hh.T * scale   # w2 = 2h fold
    Wg[0:32, NG:] = proj_w.T
    Wg[32, NG:] = proj_b
    return Wg.astype(NPBF16)


def kernel(x, W_ih, W_hh, b_ih, b_hh, proj_w, proj_b):
    x = np.asarray(x, np.float32)
    Wg = _prep_weights(np.asarray(W_ih, np.float32), np.asarray(W_hh, np.float32),
                       np.asarray(b_ih, np.float32), np.asarray(b_hh, np.float32),
                       np.asarray(proj_w, np.float32), np.asarray(proj_b, np.float32))
    ident = np.eye(128, dtype=NPBF16)
    xbf = x.astype(NPBF16).reshape(B_TOT, S_LEN * INP)

    nc = _build_nc()
    from concourse import bass_utils

    in_maps = []
    for i in range(NCORES):
        in_maps.append({
            "xt": xbf[i * B_CORE:(i + 1) * B_CORE],
            "wg": Wg,
            "ident": ident,
        })
    res = bass_utils.run_bass_kernel_spmd(nc, in_maps, core_ids=list(range(NCORES)))
    out = np.concatenate([r["out"] for r in res.results], axis=0)
    return out
